# revision 32
# baseline (speedup 1.0000x reference)
"""Masked cross-attention (B=4, NQ=100, HW=4096, D=1024, H=16) on 8 TRN2 cores.

Sharding: kv rows (keys) split 8 ways; each core runs LayerNorm + K
projection on its 512-key slice, computes masked exp(scores) and the
softmax denominators for all (b, h, q) against its keys, and triggers ONE
AllReduce of the denominators (25.6 KB) as early as possible.  The V
projection, the unnormalized context (already transposed to [d, q]) and
everything else that does not feed the denominators runs AFTER the
trigger, hidden inside the collective's peer-wait window (inter-core
launch skew ~100us dominates the collective).  After the collective a
short tail normalizes ctx^T by the global denominators (broadcast via a
tiny selector matmul) and runs the out-projection.  The host sums the 8
partial outputs and adds the folded bias.

exp(scores) for batches 2/3 stays resident in SBUF across the trigger
(batch 2 borrows the dead wq slot); batches 0/1 recompute it in the
post-trigger phase from the kept kpT/qpT (the scores matmul is
~3us/batch) because SBUF cannot hold all four.

LayerNorm gamma/beta are folded into the projection weights/biases on the
host; the V-projection bias is folded into the final output bias (exact
because softmax weights sum to one).
"""
import sys

sys.path.insert(0, "/opt/trn_rl_repo")

import numpy as np
import ml_dtypes

import concourse.bacc as bacc
import concourse.bass as bass
import concourse.mybir as mybir
import concourse.tile as tile
from concourse.bass_utils import run_bass_kernel_spmd
from concourse.masks import make_identity

B, NQ, HW, D, H = 4, 100, 4096, 1024, 16
HD = D // H          # 64
NCORE = 8
KC = HW // NCORE     # 512 keys per core per batch
NKT = KC // 128      # 4 key sub-tiles of 128
NDC = D // 128       # 8 chunks of the model dim
EPS = 1e-5
SCALE = 1.0 / np.sqrt(np.float32(HD))  # 1/8

F32 = mybir.dt.float32
BF16 = mybir.dt.bfloat16
AF = mybir.ActivationFunctionType
ALU = mybir.AluOpType

_compiled = {}


def _build():
    nc = bacc.Bacc("TRN2", target_bir_lowering=False, num_devices=NCORE)

    kv_d = nc.dram_tensor("kv", [B, NKT, 128, D], BF16, kind="ExternalInput")
    q_d = nc.dram_tensor("q", [B, NQ, D], BF16, kind="ExternalInput")
    mask_d = nc.dram_tensor("maskT", [B, 128, NKT, NQ], BF16, kind="ExternalInput")
    wq_d = nc.dram_tensor("wqT", [128, NDC, D], BF16, kind="ExternalInput")
    wk_d = nc.dram_tensor("wkT", [128, NDC, D], BF16, kind="ExternalInput")
    wv_d = nc.dram_tensor("wvT", [128, NDC, D], BF16, kind="ExternalInput")
    wo_d = nc.dram_tensor("woT", [128, NDC, D], BF16, kind="ExternalInput")
    bq_d = nc.dram_tensor("biasq", [128, NDC], F32, kind="ExternalInput")
    bk_d = nc.dram_tensor("biask", [128, NDC], F32, kind="ExternalInput")
    esel_d = nc.dram_tensor("esel", [H, NDC, 128], BF16, kind="ExternalInput")
    out_d = nc.dram_tensor("out", [128, NDC, B, NQ], F32, kind="ExternalOutput")

    with tile.TileContext(nc) as tc:
        with (
            tc.tile_pool(name="sb", bufs=1) as sb,
            tc.tile_pool(name="ps", bufs=1, space="PSUM") as ps,
            tc.tile_pool(name="dram", bufs=1, space="DRAM") as dram,
        ):
            # ---- constants ----
            ident = sb.tile([128, 128], BF16, tag="ident")
            make_identity(nc, ident[:])
            eps_t = sb.tile([128, 1], F32, tag="eps")
            nc.vector.memset(eps_t[:], EPS)
            ones_col = sb.tile([128, 1], BF16, tag="ones")
            nc.vector.memset(ones_col[:], 1.0)
            # head selectors for the recip broadcast matmuls: E[:, j, :] is
            # the [H, 128] one-hot map row h -> partitions of head h in
            # d-chunk j (h=2j rows 0:64, h=2j+1 rows 64:128); host-built.
            esel = sb.tile([H, NDC, 128], BF16, tag="esel")
            nc.scalar.dma_start(esel[:], esel_d[:])

            # Weights: [128, NDC, D] so each load is one contiguous DMA.
            # wk/wq (phase A) load on the DVE queue in parallel with kv on
            # the sync queue; wv/wo (phase B) on the scalar queue.
            wk_sb = sb.tile([128, NDC, D], BF16, tag="wk")
            wq_sb = sb.tile([128, NDC, D], BF16, tag="wq")
            wv_sb = sb.tile([128, NDC, D], BF16, tag="wv")
            wo_sb = sb.tile([128, NDC, D], BF16, tag="wo")
            bqv_sb = sb.tile([128, NDC], F32, tag="bqv")
            bkv_sb = sb.tile([128, NDC], F32, tag="bkv")
            bq_sb = [bqv_sb[:, j:j + 1] for j in range(NDC)]
            bk_sb = [bkv_sb[:, j:j + 1] for j in range(NDC)]
            nc.scalar.dma_start(wk_sb[:], wk_d[:])
            nc.scalar.dma_start(bqv_sb[:], bq_d[:])
            nc.scalar.dma_start(bkv_sb[:], bk_d[:])
            nc.scalar.dma_start(wq_sb[:], wq_d[:])
            # warm the ACT function tables (Sqrt for LN, Exp for scores)
            # while the first kv/wk DMAs are in flight.
            warm = sb.tile([128, 1], F32, tag="warm")
            nc.scalar.activation(warm[:], eps_t[:], AF.Sqrt)
            nc.scalar.activation(warm[:], eps_t[:], AF.Exp)

            def layernorm_to_bf16(x_f32, xn_bf16, p):
                """(x - mean) * rsqrt(var + eps), row-wise over the free dim."""
                stats = sb.tile([128, 2, 6], F32, tag="lnstats", bufs=4)
                nc.vector.bn_stats(stats[:p, 0, :], x_f32[:p, 0:512])
                nc.vector.bn_stats(stats[:p, 1, :], x_f32[:p, 512:1024])
                mv = sb.tile([128, 2], F32, tag="lnmv", bufs=4)
                nc.vector.bn_aggr(mv[:p], stats[:p])
                rstd = sb.tile([128, 1], F32, tag="lnrstd", bufs=4)
                nc.scalar.activation(rstd[:p], mv[:p, 1:2], AF.Sqrt, bias=eps_t[:p])
                nc.vector.reciprocal(rstd[:p], rstd[:p])
                nc.vector.tensor_scalar(
                    xn_bf16[:p], x_f32[:p], mv[:p, 0:1], rstd[:p],
                    ALU.subtract, ALU.mult,
                )

            # softmax denominators (local partial sums), DRAM side for the
            # single all-reduce.  Layout [B, H, NQ] so psum rows DMA
            # contiguously and the recip side loads [H, NQ] per batch.
            sloc = dram.tile([B, H, NQ], F32)

            qnT = sb.tile([128, NDC, B, NQ], BF16, tag="qnT")
            qpT = []
            kvnT = {}
            kpT = {}
            mask_sb = {}

            def prep_r(b, r):
                """kv load + LayerNorm + transpose of one 128-key sub-tile."""
                kvraw = sb.tile([128, D], BF16, tag="kvraw", bufs=3)
                nc.sync.dma_start(kvraw[:], kv_d[b, r])
                xn = sb.tile([128, D], BF16, tag="xn", bufs=2)
                layernorm_to_bf16(kvraw, xn, 128)
                for k4 in range(NDC // 4):
                    tr = ps.tile([128, 4, 128], BF16, tag="tr", bufs=2)
                    for kk in range(4):
                        k = 4 * k4 + kk
                        nc.tensor.transpose(
                            tr[:, kk, :], xn[:, k * 128:(k + 1) * 128], ident[:]
                        )
                    nc.vector.tensor_copy(
                        out=kvnT[b][:, 4 * k4:4 * k4 + 4, r, :], in_=tr[:]
                    )

            def kproj_half(b, half):
                """K projection of one 256-key half (r pair).  Same PE cycles
                as the full-width version (N=256 still hides LDWEIGHTS), but
                the first half starts as soon as r0/r1 are transposed, which
                removes the batch-0 PE stall while LN finishes r2/r3."""
                r0 = 2 * half
                for j in range(NDC):
                    acc = ps.tile([128, KC // 2], F32, tag="mm", bufs=3)
                    for k in range(NDC):
                        nc.tensor.matmul(
                            acc[:],
                            lhsT=wk_sb[:, k, j * 128:(j + 1) * 128],
                            rhs=kvnT[b][:, k, r0:r0 + 2, :].rearrange(
                                "p r k -> p (r k)"),
                            start=(k == 0), stop=(k == NDC - 1),
                        )
                    nc.scalar.activation(
                        kpT[b][:, j, half * 256:(half + 1) * 256], acc[:],
                        AF.Identity, bias=bk_sb[j][:],
                    )

            def prep_block(b):
                """kv load + LayerNorm + transpose + K projection for b."""
                kvnT[b] = sb.tile([128, NDC, NKT, 128], BF16, tag="kvnT", bufs=4,
                                  name=f"kvnT_{b}")
                kpT[b] = sb.tile([128, NDC, KC], BF16, tag="kpT", bufs=4,
                                 name=f"kpT_{b}")
                prep_r(b, 0)
                prep_r(b, 1)
                kproj_half(b, 0)
                prep_r(b, 2)
                prep_r(b, 3)
                kproj_half(b, 1)
                mask_sb[b] = sb.tile([128, NKT, NQ], BF16, tag="maskb", bufs=4,
                                     name=f"mask_{b}")
                nc.sync.dma_start(mask_sb[b][:], mask_d[b])

            def q_pipeline():
                """LayerNorm + transpose + projection of q, all batches."""
                for b in range(B):
                    qraw = sb.tile([NQ, D], BF16, tag="qraw", bufs=2)
                    nc.sync.dma_start(qraw[:], q_d[b])
                    qn = sb.tile([NQ, D], BF16, tag="qn", bufs=2)
                    layernorm_to_bf16(qraw, qn, NQ)
                    for k4 in range(NDC // 4):
                        tr = ps.tile([128, 4, NQ], BF16, tag="tr", bufs=2)
                        for kk in range(4):
                            k = 4 * k4 + kk
                            nc.tensor.transpose(
                                tr[:, kk, :], qn[:, k * 128:(k + 1) * 128],
                                ident[:NQ, :NQ],
                            )
                        nc.vector.tensor_copy(
                            out=qnT[:, 4 * k4:4 * k4 + 4, b, :], in_=tr[:]
                        )
                # qpT[j]: [128, B, 2, NQ] block-diagonal by head: rows 0:64
                # hold head 2j over i=0 columns, rows 64:128 hold head 2j+1
                # over i=1 columns, zeros elsewhere (so the scores matmul can
                # use the full K=128 contraction for two heads at once).
                for j in range(NDC):
                    qpT.append(
                        sb.tile([128, B, 2, NQ], BF16, tag=f"qpT{j}",
                                name=f"qpT{j}")
                    )
                    nc.gpsimd.memset(qpT[j][:], 0.0)
                    acc = ps.tile([128, B * NQ], F32, tag="sc", bufs=2)
                    for k in range(NDC):
                        nc.tensor.matmul(
                            acc[:],
                            lhsT=wq_sb[:, k, j * 128:(j + 1) * 128],
                            rhs=qnT[:, k, :, :].rearrange("p b q -> p (b q)"),
                            start=(k == 0), stop=(k == NDC - 1),
                        )
                    nc.scalar.activation(
                        qpT[j][0:HD, :, 0, :],
                        acc[0:HD, :].rearrange("p (b q) -> p b q", b=B),
                        AF.Identity, bias=bq_sb[j][0:HD],
                    )
                    nc.scalar.activation(
                        qpT[j][HD:128, :, 1, :],
                        acc[HD:128, :].rearrange("p (b q) -> p b q", b=B),
                        AF.Identity, bias=bq_sb[j][HD:128],
                    )

            def scores_exp(b, tag="exp"):
                """scores^T + exp + mask for batch b -> exp tile
                [128 keys, NKT c, H, NQ]."""
                exp_t = sb.tile([128, NKT, H, NQ], BF16, tag=tag, bufs=1,
                                name=f"exp_{b}_{tag}")
                for j in range(NDC):
                    for c2 in range(2):
                        sc = ps.tile([128, 2, 2, NQ], F32, tag="sc", bufs=2)
                        for cc in range(2):
                            c = 2 * c2 + cc
                            nc.tensor.matmul(
                                sc[:, cc, :, :].rearrange("p i q -> p (i q)"),
                                lhsT=kpT[b][:, j, c * 128:(c + 1) * 128],
                                rhs=qpT[j][:, b, :, :].rearrange(
                                    "p i q -> p (i q)"),
                                start=True, stop=True,
                            )
                        nc.scalar.activation(
                            exp_t[:, 2 * c2:2 * c2 + 2, 2 * j:2 * j + 2, :],
                            sc[:], AF.Exp, scale=float(SCALE),
                        )
                # mask multiply split across DVE and the (idle) gpsimd
                # engine: this chain gates the denominators -> collective.
                # gpsimd is ~1.8x slower per op, so it gets 5 of 16 heads.
                for h in range(H):
                    eng = nc.gpsimd if h % 3 == 2 else nc.vector
                    eng.tensor_mul(
                        exp_t[:, :, h, :], exp_t[:, :, h, :], mask_sb[b][:]
                    )
                return exp_t

            def denoms(b, exp_t):
                """Local softmax denominators for b via ones-vector matmuls;
                DMA to sloc[b] on the gpsimd queue."""
                den_sb = sb.tile([1, H, NQ], F32, tag="den", bufs=1,
                                 name=f"den_{b}")
                for h4 in range(H // 4):
                    dn = ps.tile([1, 4, NQ], F32, tag="dn", bufs=1)
                    for c in range(NKT):
                        nc.tensor.matmul(
                            dn[:].rearrange("p h q -> p (h q)"),
                            lhsT=ones_col[:],
                            rhs=exp_t[:, c, 4 * h4:4 * h4 + 4, :].rearrange(
                                "p h q -> p (h q)"),
                            start=(c == 0), stop=(c == NKT - 1),
                        )
                    nc.vector.tensor_copy(
                        out=den_sb[:, 4 * h4:4 * h4 + 4, :], in_=dn[:]
                    )
                nc.gpsimd.dma_start(sloc[b].unsqueeze(0), den_sb[:])

            def v_proj(b):
                """V projection -> vpe: [128 keys, NKT r, H, HD] (reuses the
                wk slot; wk is dead after the last K projection)."""
                vpe = sb.tile([128, NKT, H, HD], BF16, tag="wk", bufs=1,
                              name=f"vpe_{b}")
                for r in range(NKT):
                    for nh in range(2):
                        acc = ps.tile([128, 512], F32, tag="mm", bufs=3)
                        for k in range(NDC):
                            nc.tensor.matmul(
                                acc[:],
                                lhsT=kvnT[b][:, k, r, :],
                                rhs=wv_sb[:, k, nh * 512:(nh + 1) * 512],
                                start=(k == 0), stop=(k == NDC - 1),
                            )
                        nc.vector.tensor_copy(
                            out=vpe[:, r, nh * 8:(nh + 1) * 8, :],
                            in_=acc[:].rearrange("p (g d) -> p g d", g=8),
                        )
                return vpe

            # normalized-later, transposed context for all batches: [p,k,b,q]
            # (reuses the qnT slot once q is projected -- but qnT is consumed
            # only after Qproj; ctxT writes start in phase B, so alias is
            # safe by program order).  NOTE: separate tag to keep it simple.
            ctxT_all = sb.tile([128, NDC, B, NQ], BF16, tag="ctxT")

            def ctx_block(b, exp_t, vpe):
                """Unnormalized ctx^T directly in [dout, q] orientation:
                out[64 rows of head h, q] = sum_c vpe[:,c,h,:]^T @ exp."""
                for j in range(NDC):
                    cp = ps.tile([128, NQ], F32, tag="tr", bufs=2)
                    for hh in range(2):
                        h = 2 * j + hh
                        for c in range(NKT):
                            nc.tensor.matmul(
                                cp[hh * HD:(hh + 1) * HD, :],
                                lhsT=vpe[:, c, h, :],
                                rhs=exp_t[:, c, h, :],
                                start=(c == 0), stop=(c == NKT - 1),
                            )
                    nc.vector.tensor_copy(out=ctxT_all[:, j, b, :], in_=cp[:])

            # ---- schedule ----
            # Phase A: everything the denominators need, nothing else.
            prep_block(0)
            q_pipeline()
            # wv/wo (phase-B weights) load on the scalar queue, triggered
            # only once ACT reaches this point (~after b0's LN + biases) so
            # they don't steal HBM bandwidth from the critical kv+wk loads.
            nc.scalar.dma_start(wv_sb[:], wv_d[:])
            nc.scalar.dma_start(wo_sb[:], wo_d[:])
            e0_ = scores_exp(0)
            prep_block(1)
            denoms(0, e0_)
            e1_ = scores_exp(1)
            prep_block(2)
            denoms(1, e1_)
            # batch 2's exp goes into the dead wq slot, batch 3's into the
            # exp slot -- both stay alive into phase B (no recompute).
            e2_ = scores_exp(2, tag="wq")
            prep_block(3)
            denoms(2, e2_)
            e3_ = scores_exp(3)
            # fill the PE while ACT/DVE finish exp(3)+mask(3): batch 2's V
            # projection has no denominator dependency (wk is dead, its slot
            # holds vpe now).
            vpe2 = v_proj(2)
            denoms(3, e3_)

            # ONE all-reduce for all batches' denominators; peers arrive
            # late (launch skew), so phase B below runs inside the wait.
            sglob = dram.tile([B, H, NQ], F32, tag="sglob")
            nc.gpsimd.collective_compute(
                "AllReduce", ALU.add,
                replica_groups=[list(range(NCORE))],
                ins=[sloc[:].opt()], outs=[sglob[:].opt()],
            )

            # Phase B: V projection + transposed ctx; batches 2/3 reuse the
            # exp kept from phase A, batches 0/1 recompute it (cheap scores
            # matmul; SBUF can't hold all four).
            ctx_block(2, e2_, vpe2)
            vpe3 = v_proj(3)
            ctx_block(3, e3_, vpe3)
            for b in (0, 1):
                vpe = v_proj(b)
                e_t = scores_exp(b)
                ctx_block(b, e_t, vpe)

            # ---- tail (after the collective) ----
            recipT = sb.tile([H, B, NQ], F32, tag="recipT")
            nc.gpsimd.dma_start(recipT[:], sglob[:].transpose([1, 0, 2]))
            nc.vector.reciprocal(recipT[:], recipT[:])
            recipT_bf = sb.tile([H, B, NQ], BF16, tag="recipbf")
            nc.scalar.copy(recipT_bf[:], recipT[:])

            # broadcast recip rows to 64-partition head halves via the
            # selector matmuls, then normalize ctx^T in place.
            for j in range(NDC):
                rps = ps.tile([128, B, NQ], F32, tag="sc", bufs=2)
                nc.tensor.matmul(
                    rps[:].rearrange("p b q -> p (b q)"),
                    lhsT=esel[:, j, :],
                    rhs=recipT_bf[:].rearrange("p b q -> p (b q)"),
                    start=True, stop=True,
                )
                nc.vector.tensor_mul(
                    ctxT_all[:, j, :, :], ctxT_all[:, j, :, :], rps[:]
                )

            # out-projection, all 4 batches per matmul (N=400); partial
            # results stream to DRAM per output chunk (reuses the wq slot).
            out_sb = sb.tile([128, NDC, B, NQ], F32, tag="wq", bufs=1,
                             name="out_sb")
            for m in range(NDC):
                acc = ps.tile([128, B * NQ], F32, tag="sc", bufs=2)
                for k in range(NDC):
                    nc.tensor.matmul(
                        acc[:],
                        lhsT=wo_sb[:, k, m * 128:(m + 1) * 128],
                        rhs=ctxT_all[:, k, :, :].rearrange("p b q -> p (b q)"),
                        start=(k == 0), stop=(k == NDC - 1),
                    )
                nc.vector.tensor_copy(
                    out=out_sb[:, m], in_=acc[:].rearrange(
                        "p (b q) -> p b q", b=B),
                )
                nc.sync.dma_start(out_d[:, m], out_sb[:, m])

    nc.compile()
    return nc


def _prep_in_maps(q, kv, mask, in_proj_w, in_proj_b, out_w, out_b,
                  g_q, b_q, g_kv, b_kv):
    """Host-side prep: fold LN affine + V-bias, shard kv/mask per core.

    Returns (in_maps, bias_total)."""
    q = np.asarray(q, np.float32)
    kv = np.asarray(kv, np.float32)
    mask = np.asarray(mask)
    in_proj_w = np.asarray(in_proj_w, np.float32)
    in_proj_b = np.asarray(in_proj_b, np.float32)
    out_w = np.asarray(out_w, np.float32)
    out_b = np.asarray(out_b, np.float32)
    g_q = np.asarray(g_q, np.float32)
    b_q = np.asarray(b_q, np.float32)
    g_kv = np.asarray(g_kv, np.float32)
    b_kv = np.asarray(b_kv, np.float32)

    Wq, Wk, Wv = in_proj_w[:D], in_proj_w[D:2 * D], in_proj_w[2 * D:]
    bq, bk, bv = in_proj_b[:D], in_proj_b[D:2 * D], in_proj_b[2 * D:]

    # Fold LayerNorm affine into projections: LN(x)*g+b @ W^T + c
    #   = LN(x) @ (W*g)^T + (W@b + c)
    WqT = (Wq * g_q[None, :]).T.astype(ml_dtypes.bfloat16)
    WkT = (Wk * g_kv[None, :]).T.astype(ml_dtypes.bfloat16)
    WvT = (Wv * g_kv[None, :]).T.astype(ml_dtypes.bfloat16)
    bq_eff = (bq + Wq @ b_q).astype(np.float32)
    bk_eff = (bk + Wk @ b_kv).astype(np.float32)
    bv_eff = (bv + Wv @ b_kv).astype(np.float32)
    # V bias passes through softmax unchanged (weights sum to 1): fold into
    # the final output bias on the host.
    WoT = out_w.T.astype(ml_dtypes.bfloat16)
    bias_total = (out_b + out_w @ bv_eff).astype(np.float32)

    # per-query key mask; all-zero mask rows attend everywhere
    kv16 = kv.astype(ml_dtypes.bfloat16)
    allowed = (mask != 0)
    has_any = allowed.any(axis=-1, keepdims=True)
    eff = np.where(has_any, allowed, True)  # [B, NQ, HW] bool

    esel = np.zeros((H, NDC, 128), ml_dtypes.bfloat16)
    for j in range(NDC):
        esel[2 * j, j, 0:HD] = 1
        esel[2 * j + 1, j, HD:128] = 1

    common = {
        "esel": esel,
        "q": np.ascontiguousarray(q.astype(ml_dtypes.bfloat16)),
        "wqT": np.ascontiguousarray(WqT.reshape(NDC, 128, D).transpose(1, 0, 2)),
        "wkT": np.ascontiguousarray(WkT.reshape(NDC, 128, D).transpose(1, 0, 2)),
        "wvT": np.ascontiguousarray(WvT.reshape(NDC, 128, D).transpose(1, 0, 2)),
        "woT": np.ascontiguousarray(WoT.reshape(NDC, 128, D).transpose(1, 0, 2)),
        "biasq": np.ascontiguousarray(bq_eff.reshape(NDC, 128).T),
        "biask": np.ascontiguousarray(bk_eff.reshape(NDC, 128).T),
    }
    in_maps = []
    for c in range(NCORE):
        sl = slice(c * KC, (c + 1) * KC)
        kv_c = kv16[:, sl, :].reshape(B, NKT, 128, D)
        # mask slice -> [B, 128, NKT, NQ] bf16 (keysub-tile on partitions)
        m_c = eff[:, :, sl].transpose(0, 2, 1).reshape(B, NKT, 128, NQ)
        m_c = m_c.transpose(0, 2, 1, 3).astype(ml_dtypes.bfloat16)
        in_maps.append({
            **common,
            "kv": np.ascontiguousarray(kv_c),
            "maskT": np.ascontiguousarray(m_c),
        })
    return in_maps, bias_total


def kernel(q, kv, mask, in_proj_w, in_proj_b, out_w, out_b, g_q, b_q, g_kv, b_kv):
    in_maps, bias_total = _prep_in_maps(
        q, kv, mask, in_proj_w, in_proj_b, out_w, out_b, g_q, b_q, g_kv, b_kv
    )
    if "nc" not in _compiled:
        _compiled["nc"] = _build()
    nc = _compiled["nc"]

    res = run_bass_kernel_spmd(nc, in_maps, core_ids=list(range(NCORE)))

    out = np.zeros((B, NQ, D), np.float32)
    for c in range(NCORE):
        part = res.results[c]["out"]  # [128 p, NDC m, B, NQ]; dout = m*128+p
        out += part.transpose(2, 3, 1, 0).reshape(B, NQ, D)
    out += bias_total[None, None, :]
    return out


# revision 33
# speedup vs baseline: 1.0364x; 1.0364x over previous
"""Masked cross-attention (B=4, NQ=100, HW=4096, D=1024, H=16) on 8 TRN2 cores.

Sharding: kv rows (keys) split 8 ways; each core runs LayerNorm + K
projection on its 512-key slice, computes masked exp(scores) and the
softmax denominators for all (b, h, q) against its keys, and triggers ONE
AllReduce of the denominators (25.6 KB) as early as possible.  The V
projection, the unnormalized context (already transposed to [d, q]) and
everything else that does not feed the denominators runs AFTER the
trigger, hidden inside the collective's peer-wait window (inter-core
launch skew ~100us dominates the collective).  After the collective a
short tail normalizes ctx^T by the global denominators (broadcast via a
tiny selector matmul) and runs the out-projection.  The host sums the 8
partial outputs and adds the folded bias.

exp(scores) for batches 2/3 stays resident in SBUF across the trigger
(batch 2 borrows the dead wq slot); batches 0/1 recompute it in the
post-trigger phase from the kept kpT/qpT (the scores matmul is
~3us/batch) because SBUF cannot hold all four.

LayerNorm gamma/beta are folded into the projection weights/biases on the
host; the V-projection bias is folded into the final output bias (exact
because softmax weights sum to one).
"""
import sys

sys.path.insert(0, "/opt/trn_rl_repo")

import numpy as np
import ml_dtypes

import concourse.bacc as bacc
import concourse.bass as bass
import concourse.mybir as mybir
import concourse.tile as tile
from concourse.bass_utils import run_bass_kernel_spmd
from concourse.masks import make_identity

B, NQ, HW, D, H = 4, 100, 4096, 1024, 16
HD = D // H          # 64
NCORE = 8
KC = HW // NCORE     # 512 keys per core per batch
NKT = KC // 128      # 4 key sub-tiles of 128
NDC = D // 128       # 8 chunks of the model dim
EPS = 1e-5
SCALE = 1.0 / np.sqrt(np.float32(HD))  # 1/8

F32 = mybir.dt.float32
BF16 = mybir.dt.bfloat16
AF = mybir.ActivationFunctionType
ALU = mybir.AluOpType

_compiled = {}


def _build():
    nc = bacc.Bacc("TRN2", target_bir_lowering=False, num_devices=NCORE)

    kv_d = nc.dram_tensor("kv", [B, NKT, 128, D], BF16, kind="ExternalInput")
    q_d = nc.dram_tensor("q", [B, NQ, D], BF16, kind="ExternalInput")
    mask_d = nc.dram_tensor("maskT", [B, 128, NKT, NQ], BF16, kind="ExternalInput")
    wq_d = nc.dram_tensor("wqT", [128, NDC, D], BF16, kind="ExternalInput")
    wk_d = nc.dram_tensor("wkT", [128, NDC, D], BF16, kind="ExternalInput")
    wv_d = nc.dram_tensor("wvT", [128, NDC, D], BF16, kind="ExternalInput")
    wo_d = nc.dram_tensor("woT", [128, NDC, D], BF16, kind="ExternalInput")
    bq_d = nc.dram_tensor("biasq", [128, NDC], F32, kind="ExternalInput")
    bk_d = nc.dram_tensor("biask", [128, NDC], F32, kind="ExternalInput")
    esel_d = nc.dram_tensor("esel", [H, NDC, 128], BF16, kind="ExternalInput")
    out_d = nc.dram_tensor("out", [128, NDC, B, NQ], F32, kind="ExternalOutput")

    with tile.TileContext(nc) as tc:
        with (
            tc.tile_pool(name="sb", bufs=1) as sb,
            tc.tile_pool(name="ps", bufs=1, space="PSUM") as ps,
            tc.tile_pool(name="dram", bufs=1, space="DRAM") as dram,
        ):
            # ---- constants ----
            ident = sb.tile([128, 128], BF16, tag="ident")
            make_identity(nc, ident[:])
            eps_t = sb.tile([128, 1], F32, tag="eps")
            nc.vector.memset(eps_t[:], EPS)
            ones_col = sb.tile([128, 1], BF16, tag="ones")
            nc.vector.memset(ones_col[:], 1.0)
            # head selectors for the recip broadcast matmuls: E[:, j, :] is
            # the [H, 128] one-hot map row h -> partitions of head h in
            # d-chunk j (h=2j rows 0:64, h=2j+1 rows 64:128); host-built.
            esel = sb.tile([H, NDC, 128], BF16, tag="esel")
            nc.scalar.dma_start(esel[:], esel_d[:])

            # Weights: [128, NDC, D] so each load is one contiguous DMA.
            # wk/wq (phase A) load on the DVE queue in parallel with kv on
            # the sync queue; wv/wo (phase B) on the scalar queue.
            wk_sb = sb.tile([128, NDC, D], BF16, tag="wk")
            wq_sb = sb.tile([128, NDC, D], BF16, tag="wq")
            wv_sb = sb.tile([128, NDC, D], BF16, tag="wv")
            wo_sb = sb.tile([128, NDC, D], BF16, tag="wo")
            bqv_sb = sb.tile([128, NDC], F32, tag="bqv")
            bkv_sb = sb.tile([128, NDC], F32, tag="bkv")
            bq_sb = [bqv_sb[:, j:j + 1] for j in range(NDC)]
            bk_sb = [bkv_sb[:, j:j + 1] for j in range(NDC)]
            nc.scalar.dma_start(wk_sb[:], wk_d[:])
            nc.scalar.dma_start(bqv_sb[:], bq_d[:])
            nc.scalar.dma_start(bkv_sb[:], bk_d[:])
            nc.scalar.dma_start(wq_sb[:], wq_d[:])
            # warm the ACT function tables (Sqrt for LN, Exp for scores)
            # while the first kv/wk DMAs are in flight.
            warm = sb.tile([128, 1], F32, tag="warm")
            nc.scalar.activation(warm[:], eps_t[:], AF.Sqrt)
            nc.scalar.activation(warm[:], eps_t[:], AF.Exp)

            def layernorm_to_bf16(x_f32, xn_bf16, p):
                """(x - mean) * rsqrt(var + eps), row-wise over the free dim."""
                stats = sb.tile([128, 2, 6], F32, tag="lnstats", bufs=4)
                nc.vector.bn_stats(stats[:p, 0, :], x_f32[:p, 0:512])
                nc.vector.bn_stats(stats[:p, 1, :], x_f32[:p, 512:1024])
                mv = sb.tile([128, 2], F32, tag="lnmv", bufs=4)
                nc.vector.bn_aggr(mv[:p], stats[:p])
                rstd = sb.tile([128, 1], F32, tag="lnrstd", bufs=4)
                nc.scalar.activation(rstd[:p], mv[:p, 1:2], AF.Sqrt, bias=eps_t[:p])
                nc.vector.reciprocal(rstd[:p], rstd[:p])
                nc.vector.tensor_scalar(
                    xn_bf16[:p], x_f32[:p], mv[:p, 0:1], rstd[:p],
                    ALU.subtract, ALU.mult,
                )

            # softmax denominators (local partial sums), DRAM side for the
            # single all-reduce.  Layout [B, H, NQ] so psum rows DMA
            # contiguously and the recip side loads [H, NQ] per batch.
            sloc = dram.tile([B, H, NQ], F32)

            qnT = sb.tile([128, NDC, B, NQ], BF16, tag="qnT")
            qpT = []
            kvnT = {}
            kpT = {}
            mask_sb = {}

            def prep_r(b, r):
                """kv load + LayerNorm + transpose of one 128-key sub-tile."""
                kvraw = sb.tile([128, D], BF16, tag="kvraw", bufs=3)
                nc.sync.dma_start(kvraw[:], kv_d[b, r])
                xn = sb.tile([128, D], BF16, tag="xn", bufs=2)
                layernorm_to_bf16(kvraw, xn, 128)
                for k4 in range(NDC // 4):
                    tr = ps.tile([128, 4, 128], BF16, tag="tr", bufs=2)
                    for kk in range(4):
                        k = 4 * k4 + kk
                        nc.tensor.transpose(
                            tr[:, kk, :], xn[:, k * 128:(k + 1) * 128], ident[:]
                        )
                    nc.vector.tensor_copy(
                        out=kvnT[b][:, 4 * k4:4 * k4 + 4, r, :], in_=tr[:]
                    )

            def kproj_half(b, half):
                """K projection of one 256-key half (r pair).  Same PE cycles
                as the full-width version (N=256 still hides LDWEIGHTS), but
                the first half starts as soon as r0/r1 are transposed, which
                removes the batch-0 PE stall while LN finishes r2/r3."""
                r0 = 2 * half
                for j in range(NDC):
                    acc = ps.tile([128, KC // 2], F32, tag="mm", bufs=3)
                    for k in range(NDC):
                        nc.tensor.matmul(
                            acc[:],
                            lhsT=wk_sb[:, k, j * 128:(j + 1) * 128],
                            rhs=kvnT[b][:, k, r0:r0 + 2, :].rearrange(
                                "p r k -> p (r k)"),
                            start=(k == 0), stop=(k == NDC - 1),
                        )
                    nc.scalar.activation(
                        kpT[b][:, j, half * 256:(half + 1) * 256], acc[:],
                        AF.Identity, bias=bk_sb[j][:],
                    )

            def prep_block(b):
                """kv load + LayerNorm + transpose + K projection for b."""
                kvnT[b] = sb.tile([128, NDC, NKT, 128], BF16, tag="kvnT", bufs=4,
                                  name=f"kvnT_{b}")
                kpT[b] = sb.tile([128, NDC, KC], BF16, tag="kpT", bufs=4,
                                 name=f"kpT_{b}")
                prep_r(b, 0)
                prep_r(b, 1)
                kproj_half(b, 0)
                prep_r(b, 2)
                prep_r(b, 3)
                kproj_half(b, 1)
                mask_sb[b] = sb.tile([128, NKT, NQ], BF16, tag="maskb", bufs=4,
                                     name=f"mask_{b}")
                nc.sync.dma_start(mask_sb[b][:], mask_d[b])

            def q_pipeline():
                """LayerNorm + transpose + projection of q, all batches."""
                for b in range(B):
                    # gpsimd queue: idle early, so the q loads neither wait
                    # behind kv loads on the sync queue nor on slot reuse
                    qraw = sb.tile([NQ, D], BF16, tag="qraw", bufs=4)
                    nc.gpsimd.dma_start(qraw[:], q_d[b])
                    qn = sb.tile([NQ, D], BF16, tag="qn", bufs=2)
                    layernorm_to_bf16(qraw, qn, NQ)
                    for k4 in range(NDC // 4):
                        tr = ps.tile([128, 4, NQ], BF16, tag="tr", bufs=2)
                        for kk in range(4):
                            k = 4 * k4 + kk
                            nc.tensor.transpose(
                                tr[:, kk, :], qn[:, k * 128:(k + 1) * 128],
                                ident[:NQ, :NQ],
                            )
                        nc.vector.tensor_copy(
                            out=qnT[:, 4 * k4:4 * k4 + 4, b, :], in_=tr[:]
                        )
                # qpT[j]: [128, B, 2, NQ] block-diagonal by head: rows 0:64
                # hold head 2j over i=0 columns, rows 64:128 hold head 2j+1
                # over i=1 columns, zeros elsewhere (so the scores matmul can
                # use the full K=128 contraction for two heads at once).
                for j in range(NDC):
                    qpT.append(
                        sb.tile([128, B, 2, NQ], BF16, tag=f"qpT{j}",
                                name=f"qpT{j}")
                    )
                    nc.gpsimd.memset(qpT[j][:], 0.0)
                    acc = ps.tile([128, B * NQ], F32, tag="sc", bufs=2)
                    for k in range(NDC):
                        nc.tensor.matmul(
                            acc[:],
                            lhsT=wq_sb[:, k, j * 128:(j + 1) * 128],
                            rhs=qnT[:, k, :, :].rearrange("p b q -> p (b q)"),
                            start=(k == 0), stop=(k == NDC - 1),
                        )
                    nc.scalar.activation(
                        qpT[j][0:HD, :, 0, :],
                        acc[0:HD, :].rearrange("p (b q) -> p b q", b=B),
                        AF.Identity, bias=bq_sb[j][0:HD],
                    )
                    nc.scalar.activation(
                        qpT[j][HD:128, :, 1, :],
                        acc[HD:128, :].rearrange("p (b q) -> p b q", b=B),
                        AF.Identity, bias=bq_sb[j][HD:128],
                    )

            def scores_exp(b, tag="exp"):
                """scores^T + exp + mask for batch b -> exp tile
                [128 keys, NKT c, H, NQ]."""
                exp_t = sb.tile([128, NKT, H, NQ], BF16, tag=tag, bufs=1,
                                name=f"exp_{b}_{tag}")
                for j in range(NDC):
                    for c2 in range(2):
                        sc = ps.tile([128, 2, 2, NQ], F32, tag="sc", bufs=2)
                        for cc in range(2):
                            c = 2 * c2 + cc
                            nc.tensor.matmul(
                                sc[:, cc, :, :].rearrange("p i q -> p (i q)"),
                                lhsT=kpT[b][:, j, c * 128:(c + 1) * 128],
                                rhs=qpT[j][:, b, :, :].rearrange(
                                    "p i q -> p (i q)"),
                                start=True, stop=True,
                            )
                        nc.scalar.activation(
                            exp_t[:, 2 * c2:2 * c2 + 2, 2 * j:2 * j + 2, :],
                            sc[:], AF.Exp, scale=float(SCALE),
                        )
                # mask multiply split across DVE and the (idle) gpsimd
                # engine: this chain gates the denominators -> collective.
                # gpsimd is ~1.8x slower per op, so it gets 5 of 16 heads.
                for h in range(H):
                    eng = nc.gpsimd if h % 3 == 2 else nc.vector
                    eng.tensor_mul(
                        exp_t[:, :, h, :], exp_t[:, :, h, :], mask_sb[b][:]
                    )
                return exp_t

            def denoms(b, exp_t):
                """Local softmax denominators for b via ones-vector matmuls;
                DMA to sloc[b] on the gpsimd queue."""
                den_sb = sb.tile([1, H, NQ], F32, tag="den", bufs=1,
                                 name=f"den_{b}")
                for h4 in range(H // 4):
                    dn = ps.tile([1, 4, NQ], F32, tag="dn", bufs=1)
                    for c in range(NKT):
                        nc.tensor.matmul(
                            dn[:].rearrange("p h q -> p (h q)"),
                            lhsT=ones_col[:],
                            rhs=exp_t[:, c, 4 * h4:4 * h4 + 4, :].rearrange(
                                "p h q -> p (h q)"),
                            start=(c == 0), stop=(c == NKT - 1),
                        )
                    nc.vector.tensor_copy(
                        out=den_sb[:, 4 * h4:4 * h4 + 4, :], in_=dn[:]
                    )
                nc.gpsimd.dma_start(sloc[b].unsqueeze(0), den_sb[:])

            def v_proj(b):
                """V projection -> vpe: [128 keys, NKT r, H, HD] (reuses the
                wk slot; wk is dead after the last K projection)."""
                vpe = sb.tile([128, NKT, H, HD], BF16, tag="wk", bufs=1,
                              name=f"vpe_{b}")
                for r in range(NKT):
                    for nh in range(2):
                        acc = ps.tile([128, 512], F32, tag="mm", bufs=3)
                        for k in range(NDC):
                            nc.tensor.matmul(
                                acc[:],
                                lhsT=kvnT[b][:, k, r, :],
                                rhs=wv_sb[:, k, nh * 512:(nh + 1) * 512],
                                start=(k == 0), stop=(k == NDC - 1),
                            )
                        nc.vector.tensor_copy(
                            out=vpe[:, r, nh * 8:(nh + 1) * 8, :],
                            in_=acc[:].rearrange("p (g d) -> p g d", g=8),
                        )
                return vpe

            # normalized-later, transposed context for all batches: [p,k,b,q]
            # (reuses the qnT slot once q is projected -- but qnT is consumed
            # only after Qproj; ctxT writes start in phase B, so alias is
            # safe by program order).  NOTE: separate tag to keep it simple.
            ctxT_all = sb.tile([128, NDC, B, NQ], BF16, tag="ctxT")

            def ctx_block(b, exp_t, vpe):
                """Unnormalized ctx^T directly in [dout, q] orientation:
                out[64 rows of head h, q] = sum_c vpe[:,c,h,:]^T @ exp."""
                for j in range(NDC):
                    cp = ps.tile([128, NQ], F32, tag="tr", bufs=2)
                    for hh in range(2):
                        h = 2 * j + hh
                        for c in range(NKT):
                            nc.tensor.matmul(
                                cp[hh * HD:(hh + 1) * HD, :],
                                lhsT=vpe[:, c, h, :],
                                rhs=exp_t[:, c, h, :],
                                start=(c == 0), stop=(c == NKT - 1),
                            )
                    nc.vector.tensor_copy(out=ctxT_all[:, j, b, :], in_=cp[:])

            # ---- schedule ----
            # Phase A: everything the denominators need, nothing else.
            prep_block(0)
            q_pipeline()
            # wv/wo (phase-B weights) load on the scalar queue, triggered
            # only once ACT reaches this point (~after b0's LN + biases) so
            # they don't steal HBM bandwidth from the critical kv+wk loads.
            nc.scalar.dma_start(wv_sb[:], wv_d[:])
            nc.scalar.dma_start(wo_sb[:], wo_d[:])
            e0_ = scores_exp(0)
            prep_block(1)
            denoms(0, e0_)
            e1_ = scores_exp(1)
            prep_block(2)
            denoms(1, e1_)
            # batch 2's exp goes into the dead wq slot, batch 3's into the
            # exp slot -- both stay alive into phase B (no recompute).
            e2_ = scores_exp(2, tag="wq")
            prep_block(3)
            denoms(2, e2_)
            e3_ = scores_exp(3)
            # fill the PE while ACT/DVE finish exp(3)+mask(3): batch 2's V
            # projection has no denominator dependency (wk is dead, its slot
            # holds vpe now).
            vpe2 = v_proj(2)
            denoms(3, e3_)

            # ONE all-reduce for all batches' denominators; peers arrive
            # late (launch skew), so phase B below runs inside the wait.
            sglob = dram.tile([B, H, NQ], F32, tag="sglob")
            nc.gpsimd.collective_compute(
                "AllReduce", ALU.add,
                replica_groups=[list(range(NCORE))],
                ins=[sloc[:].opt()], outs=[sglob[:].opt()],
            )

            # Phase B: V projection + transposed ctx; batches 2/3 reuse the
            # exp kept from phase A, batches 0/1 recompute it (cheap scores
            # matmul; SBUF can't hold all four).
            ctx_block(2, e2_, vpe2)
            vpe3 = v_proj(3)
            ctx_block(3, e3_, vpe3)
            for b in (0, 1):
                vpe = v_proj(b)
                e_t = scores_exp(b)
                ctx_block(b, e_t, vpe)

            # ---- tail (after the collective) ----
            recipT = sb.tile([H, B, NQ], F32, tag="recipT")
            nc.gpsimd.dma_start(recipT[:], sglob[:].transpose([1, 0, 2]))
            nc.vector.reciprocal(recipT[:], recipT[:])
            recipT_bf = sb.tile([H, B, NQ], BF16, tag="recipbf")
            nc.scalar.copy(recipT_bf[:], recipT[:])

            # broadcast recip rows to 64-partition head halves via the
            # selector matmuls, then normalize ctx^T in place.
            for j in range(NDC):
                rps = ps.tile([128, B, NQ], F32, tag="sc", bufs=2)
                nc.tensor.matmul(
                    rps[:].rearrange("p b q -> p (b q)"),
                    lhsT=esel[:, j, :],
                    rhs=recipT_bf[:].rearrange("p b q -> p (b q)"),
                    start=True, stop=True,
                )
                nc.vector.tensor_mul(
                    ctxT_all[:, j, :, :], ctxT_all[:, j, :, :], rps[:]
                )

            # out-projection, all 4 batches per matmul (N=400); partial
            # results stream to DRAM per output chunk (reuses the wq slot).
            out_sb = sb.tile([128, NDC, B, NQ], F32, tag="wq", bufs=1,
                             name="out_sb")
            for m in range(NDC):
                acc = ps.tile([128, B * NQ], F32, tag="sc", bufs=2)
                for k in range(NDC):
                    nc.tensor.matmul(
                        acc[:],
                        lhsT=wo_sb[:, k, m * 128:(m + 1) * 128],
                        rhs=ctxT_all[:, k, :, :].rearrange("p b q -> p (b q)"),
                        start=(k == 0), stop=(k == NDC - 1),
                    )
                nc.vector.tensor_copy(
                    out=out_sb[:, m], in_=acc[:].rearrange(
                        "p (b q) -> p b q", b=B),
                )
                nc.sync.dma_start(out_d[:, m], out_sb[:, m])

    nc.compile()
    return nc


def _prep_in_maps(q, kv, mask, in_proj_w, in_proj_b, out_w, out_b,
                  g_q, b_q, g_kv, b_kv):
    """Host-side prep: fold LN affine + V-bias, shard kv/mask per core.

    Returns (in_maps, bias_total)."""
    q = np.asarray(q, np.float32)
    kv = np.asarray(kv, np.float32)
    mask = np.asarray(mask)
    in_proj_w = np.asarray(in_proj_w, np.float32)
    in_proj_b = np.asarray(in_proj_b, np.float32)
    out_w = np.asarray(out_w, np.float32)
    out_b = np.asarray(out_b, np.float32)
    g_q = np.asarray(g_q, np.float32)
    b_q = np.asarray(b_q, np.float32)
    g_kv = np.asarray(g_kv, np.float32)
    b_kv = np.asarray(b_kv, np.float32)

    Wq, Wk, Wv = in_proj_w[:D], in_proj_w[D:2 * D], in_proj_w[2 * D:]
    bq, bk, bv = in_proj_b[:D], in_proj_b[D:2 * D], in_proj_b[2 * D:]

    # Fold LayerNorm affine into projections: LN(x)*g+b @ W^T + c
    #   = LN(x) @ (W*g)^T + (W@b + c)
    WqT = (Wq * g_q[None, :]).T.astype(ml_dtypes.bfloat16)
    WkT = (Wk * g_kv[None, :]).T.astype(ml_dtypes.bfloat16)
    WvT = (Wv * g_kv[None, :]).T.astype(ml_dtypes.bfloat16)
    bq_eff = (bq + Wq @ b_q).astype(np.float32)
    bk_eff = (bk + Wk @ b_kv).astype(np.float32)
    bv_eff = (bv + Wv @ b_kv).astype(np.float32)
    # V bias passes through softmax unchanged (weights sum to 1): fold into
    # the final output bias on the host.
    WoT = out_w.T.astype(ml_dtypes.bfloat16)
    bias_total = (out_b + out_w @ bv_eff).astype(np.float32)

    # per-query key mask; all-zero mask rows attend everywhere
    kv16 = kv.astype(ml_dtypes.bfloat16)
    allowed = (mask != 0)
    has_any = allowed.any(axis=-1, keepdims=True)
    eff = np.where(has_any, allowed, True)  # [B, NQ, HW] bool

    esel = np.zeros((H, NDC, 128), ml_dtypes.bfloat16)
    for j in range(NDC):
        esel[2 * j, j, 0:HD] = 1
        esel[2 * j + 1, j, HD:128] = 1

    common = {
        "esel": esel,
        "q": np.ascontiguousarray(q.astype(ml_dtypes.bfloat16)),
        "wqT": np.ascontiguousarray(WqT.reshape(NDC, 128, D).transpose(1, 0, 2)),
        "wkT": np.ascontiguousarray(WkT.reshape(NDC, 128, D).transpose(1, 0, 2)),
        "wvT": np.ascontiguousarray(WvT.reshape(NDC, 128, D).transpose(1, 0, 2)),
        "woT": np.ascontiguousarray(WoT.reshape(NDC, 128, D).transpose(1, 0, 2)),
        "biasq": np.ascontiguousarray(bq_eff.reshape(NDC, 128).T),
        "biask": np.ascontiguousarray(bk_eff.reshape(NDC, 128).T),
    }
    in_maps = []
    for c in range(NCORE):
        sl = slice(c * KC, (c + 1) * KC)
        kv_c = kv16[:, sl, :].reshape(B, NKT, 128, D)
        # mask slice -> [B, 128, NKT, NQ] bf16 (keysub-tile on partitions)
        m_c = eff[:, :, sl].transpose(0, 2, 1).reshape(B, NKT, 128, NQ)
        m_c = m_c.transpose(0, 2, 1, 3).astype(ml_dtypes.bfloat16)
        in_maps.append({
            **common,
            "kv": np.ascontiguousarray(kv_c),
            "maskT": np.ascontiguousarray(m_c),
        })
    return in_maps, bias_total


def kernel(q, kv, mask, in_proj_w, in_proj_b, out_w, out_b, g_q, b_q, g_kv, b_kv):
    in_maps, bias_total = _prep_in_maps(
        q, kv, mask, in_proj_w, in_proj_b, out_w, out_b, g_q, b_q, g_kv, b_kv
    )
    if "nc" not in _compiled:
        _compiled["nc"] = _build()
    nc = _compiled["nc"]

    res = run_bass_kernel_spmd(nc, in_maps, core_ids=list(range(NCORE)))

    out = np.zeros((B, NQ, D), np.float32)
    for c in range(NCORE):
        part = res.results[c]["out"]  # [128 p, NDC m, B, NQ]; dout = m*128+p
        out += part.transpose(2, 3, 1, 0).reshape(B, NQ, D)
    out += bias_total[None, None, :]
    return out
